# revision 32
# baseline (speedup 1.0000x reference)
"""AttentionSubsample kernel for 8 trn2 NeuronCores (head-parallel).

Sharding: 8 heads -> 8 cores; each core runs its head through attn@v and a
64-channel slice of the output projection after a per-chunk AllGather.

Engine assignment (from trace-driven iteration against the timeline model):
- All softmax exps on ACT (table exp, PSUM->bf16); splitting exp onto DVE
  (Schraudolph bit-trick, machinery still present under D_SET) measured
  slower due to cross-engine psA-slot stalls.
- exp(bias) multiplies and softmax/hardswish drain chain on DVE; PSUM
  drains of projections on ACT (Copy shares the exp table set, no reload);
  v transposed on the PE via identity matmuls (no DMA-queue latency).
- Asymmetric q-chunks (512, 512, 320) front-load ACT work so the tail
  AllGather is small and starts early; per-chunk collectives stay off each
  other's critical path; per-b bounce DMAs overlap the last drain.
- Activation-table loads hoisted via dummy Sqrt/Exp activations.
- All matmuls bf16 (fp8 QK/AV measured numerically unsafe here).
"""

import numpy as np
import ml_dtypes

import concourse.bass as bass
import concourse.mybir as mybir
import concourse.tile as tile
from concourse import bacc
from contextlib import ExitStack
from concourse.bass_utils import run_bass_kernel_spmd

BF16 = mybir.dt.bfloat16
F32 = mybir.dt.float32
I16 = mybir.dt.int16
bf16 = ml_dtypes.bfloat16

B = 2
ROW, COL = 63, 84
ROW_, COL_ = 32, 42
N = ROW * COL            # 5292 kv tokens
NQ = ROW_ * COL_         # 1344 q tokens
NPAD = 5376              # 42*128 padded kv tokens
KT = NPAD // 128         # 42 k-tiles
QC = 448                 # q chunk for projections / bn_stats
NQC = NQ // QC           # 3
# asymmetric attention q-chunks: front-load the work so the last chunk's
# AllGather (the tail) is small and starts early
QCS = (512, 512, 320)
QCO = (0, 512, 1024)
assert sum(QCS) == NQ
CIN = 256
H = 8
KD = 16
KDE = KD + 1             # +1 schraudolph-const contract row
DV = 32
HKV = KD + DV
KVP = 64                 # padded kv rows: k at 0:16, v at 32:64
OC = 64                  # per-core slice of the 512 output channels
GRP = 3                  # k-tiles per exp group
NGRP = KT // GRP         # 14
EPS = 1e-5
SCALE = KD ** -0.5
NCORES = 8

# --- engine-assignment knobs -------------------------------------------------
# group path assignment: D-groups (DVE schraudolph) interleaved among
# A-groups (ACT exp + DVE/Pool eb-mult); POOL_D run schraudolph on Pool.
D_SET = frozenset()             # DVE/Pool schraudolph groups (empty: all-ACT
                                # exp won in the sweep - no cross-engine stalls)
POOL_D = frozenset()            # subset of D_SET handled by Pool
POOL_MULT = frozenset()         # A-groups whose eb-mult runs on Pool
# process order per (qc): D-groups spread early-but-not-first so DVE's
# prologue/drain leftovers drain while ACT chews A-units (AV accumulation
# commutes, so any order is valid)
PROC_ORDER = list(range(NGRP))
assert sorted(PROC_ORDER) == list(range(NGRP))
POOL_DRAIN = False              # hardswish drain chain on Pool instead of DVE
# schraudolph constants
LOG2E = 1.4426950408889634
SCH_C = 0.0450466
ASCH = 128.0 * LOG2E * SCALE          # folded into D-range kT scale
QCONST = 16256.0                      # 128*127, exactly representable in bf16
EBB_SCALE = 128.0 * LOG2E             # bias -> bits
EBB_SHIFT = -128.0 * SCH_C            # -5.766, folded into ebb host-side

SPBUFS = 6
EBBUFS = 7

LAST_EXEC_NS = None
_prog_cache = {}


def _build_program():
    nc = bacc.Bacc(num_devices=NCORES)

    xT = nc.dram_tensor("xT", [B, 2, 128, NPAD], BF16, kind="ExternalInput")
    wkvT = nc.dram_tensor("wkvT", [2, 128, KVP], BF16, kind="ExternalInput")
    wqT = nc.dram_tensor("wqT", [2, 128, KD], BF16, kind="ExternalInput")
    wpT = nc.dram_tensor("wpT", [2, 128, OC], BF16, kind="ExternalInput")
    kv_gb = nc.dram_tensor("kv_gb", [KVP, 4], F32, kind="ExternalInput")
    q_gb = nc.dram_tensor("q_gb", [KD, 2], F32, kind="ExternalInput")
    p_gb = nc.dram_tensor("p_gb", [OC, 2], F32, kind="ExternalInput")
    ebT = nc.dram_tensor("ebT", [NGRP, 128, GRP, NQ], BF16,
                         kind="ExternalInput")
    krowT = nc.dram_tensor("krowT", [NPAD], BF16, kind="ExternalInput")
    identT = nc.dram_tensor("identT", [DV, DV], BF16, kind="ExternalInput")
    qrowT = nc.dram_tensor("qrowT", [NQ], BF16, kind="ExternalInput")
    yT = nc.dram_tensor("yT", [OC, B * NQ], F32, kind="ExternalOutput")

    with ExitStack() as ctx:
        tc = ctx.enter_context(tile.TileContext(nc))
        const = ctx.enter_context(tc.tile_pool(name="const", bufs=1))
        big = ctx.enter_context(tc.tile_pool(name="big", bufs=1))
        vtp = ctx.enter_context(tc.tile_pool(name="vtp", bufs=1))
        vtp2 = ctx.enter_context(tc.tile_pool(name="vtp2", bufs=2))
        spool = ctx.enter_context(tc.tile_pool(name="spool", bufs=SPBUFS))
        ebpool = ctx.enter_context(tc.tile_pool(name="ebpool", bufs=EBBUFS))
        small = ctx.enter_context(tc.tile_pool(name="small", bufs=4))
        drain = ctx.enter_context(tc.tile_pool(name="drain", bufs=2))
        psA = ctx.enter_context(tc.tile_pool(name="psA", bufs=2, space="PSUM"))
        psB = ctx.enter_context(tc.tile_pool(name="psB", bufs=2, space="PSUM"))
        dram = ctx.enter_context(tc.tile_pool(name="dram", bufs=4, space="DRAM"))

        mult = mybir.AluOpType.mult
        add = mybir.AluOpType.add
        amin = mybir.AluOpType.min
        Act = mybir.ActivationFunctionType

        # ------------------------- load inputs -------------------------
        xt_sb = big.tile([128, B, 2, NPAD], BF16, tag="xt")
        wkv_sb = const.tile([128, 2, KVP], BF16, tag="wkv")
        wq_sb = const.tile([128, 2, KD], BF16, tag="wq")
        wp_sb = const.tile([128, 2, OC], BF16, tag="wp")
        for c in range(2):
            nc.sync.dma_start(out=wkv_sb[:, c, :], in_=wkvT[c])
            nc.sync.dma_start(out=wq_sb[:, c, :], in_=wqT[c])
            nc.sync.dma_start(out=wp_sb[:, c, :], in_=wpT[c])
        # quarter-tensor x loads (best in the granularity sweep)
        XCH = NPAD // 4
        for t in range(4):
            for b in range(B):
                for c in range(2):
                    nc.sync.dma_start(out=xt_sb[:, b, c, bass.ts(t, XCH)],
                                      in_=xT[b, c, :, bass.ts(t, XCH)])
        kvgb_sb = const.tile([KVP, 4], F32, tag="kvgb")
        qgb_sb = const.tile([KD, 2], F32, tag="qgb")
        pgb_sb = const.tile([OC, 2], F32, tag="pgb")
        nc.sync.dma_start(out=kvgb_sb, in_=kv_gb[:, :])
        nc.sync.dma_start(out=qgb_sb, in_=q_gb[:, :])
        nc.sync.dma_start(out=pgb_sb, in_=p_gb[:, :])
        ident32 = const.tile([DV, DV], BF16, tag="ident32")
        nc.sync.dma_start(out=ident32, in_=identT[:, :])
        eps_t = const.tile([128, 1], F32, tag="eps")
        nc.vector.memset(eps_t, EPS)
        dummy_t = const.tile([1, 1], F32, tag="dummy")
        nc.vector.memset(dummy_t, 1.0)
        # force the sqrt-table load at t=0 (copy lives in every set, so the
        # prologue PSUM copies don't need another load)
        nc.scalar.activation(out=dummy_t, in_=dummy_t, func=Act.Sqrt)
        ones1_t = const.tile([1, DV], F32, tag="ones1")
        nc.vector.memset(ones1_t, 1.0)

        # ------------------------- projections -------------------------
        TCH = 448
        NT_KV = NPAD // TCH   # 12
        # projections per batch; q-proj reads the subsample via a strided
        # view of xt (no separate xs DMA); stats interleaved
        QPC = 336
        y_q = big.tile([KD, B, NQ], BF16, tag="yq")
        st_q = small.tile([KD, 2 * 4, 6], F32, tag="st_q")
        y_kv = big.tile([KVP, B, NPAD], BF16, tag="ykv")
        st_kv = small.tile([KVP, 2 * NT_KV, 6], F32, tag="st_kv")
        for t4 in range(4):
            for b in range(B):
                for t in range(3 * t4, 3 * t4 + 3):
                    ps = psB.tile([KVP, TCH], F32, tag="ps_small")
                    for c in range(2):
                        nc.tensor.matmul(ps, wkv_sb[:, c, :],
                                         xt_sb[:, b, c, bass.ts(t, TCH)],
                                         start=(c == 0), stop=(c == 1))
                    nc.scalar.copy(out=y_kv[:, b, bass.ts(t, TCH)], in_=ps)
                    if t > 0:
                        i = t - 1
                        nc.vector.bn_stats(out=st_kv[:, b * NT_KV + i, :],
                                           in_=y_kv[:, b, bass.ds(i * 441, 441)])
            for b in range(B):
                t = t4
                ps = psB.tile([KD, QPC], F32, tag="ps_small")
                for c in range(2):
                    xsv = xt_sb[:, b, c, 0:N].rearrange(
                        "p (r c2) -> p r c2",
                        r=ROW)[:, 16 * t:min(16 * (t + 1), ROW):2, ::2]
                    nc.tensor.matmul(ps, wq_sb[:, c, :], xsv,
                                     start=(c == 0), stop=(c == 1))
                nc.scalar.copy(out=y_q[:, b, bass.ts(t, QPC)], in_=ps)
                nc.vector.bn_stats(out=st_q[:, b * 4 + t, :],
                                   in_=y_q[:, b, bass.ts(t, QPC)])
        for b in range(B):
            nc.vector.bn_stats(out=st_kv[:, b * NT_KV + NT_KV - 1, :],
                               in_=y_kv[:, b, bass.ds(11 * 441, 441)])

        # ------------------------- batch norms -------------------------
        def bn_scale_shift(mv, gb, P, name):
            s = small.tile([P, 1], F32, tag=f"s_{name}")
            t = small.tile([P, 1], F32, tag=f"t_{name}")
            nc.scalar.activation(out=s, in_=mv[:, 1:2], func=Act.Sqrt,
                                 bias=eps_t[0:P])
            nc.vector.reciprocal(out=s, in_=s)
            nc.vector.tensor_mul(s, s, gb[:, 0:1])
            nc.vector.tensor_mul(t, mv[:, 0:1], s)
            nc.vector.tensor_scalar(out=t, in0=t, scalar1=-1.0, scalar2=None,
                                    op0=mult)
            nc.vector.tensor_add(t, t, gb[:, 1:2])
            return s, t

        mv_q = small.tile([KD, 2], F32, tag="mv_q")
        nc.vector.bn_aggr(out=mv_q, in_=st_q)
        s_q, t_q = bn_scale_shift(mv_q, qgb_sb, KD, "q")

        kT = big.tile([KDE, B, NPAD], BF16, tag="kT")
        qT = big.tile([KDE, B, NQ], BF16, tag="qT")
        v_aug = big.tile([128, B, KT, DV + 1], BF16, tag="vaug")
        # qT norm early (only needs q stats); b1 on ACT
        nc.vector.tensor_scalar(out=qT[0:KD, 0, :], in0=y_q[:, 0, :],
                                scalar1=s_q, scalar2=t_q,
                                op0=mult, op1=add)
        nc.scalar.activation(out=qT[0:KD, 1, :], in_=y_q[:, 1, :],
                             func=Act.Identity, scale=s_q, bias=t_q)
        for b in range(B):
            nc.sync.dma_start(out=qT[KD:KDE, b, :], in_=qrowT[:])

        mv_kv = small.tile([KVP, 2], F32, tag="mv_kv")
        nc.vector.bn_aggr(out=mv_kv, in_=st_kv)
        s_kv, t_kv = bn_scale_shift(mv_kv, kvgb_sb, KVP, "kv")
        nc.scalar.activation(out=dummy_t, in_=dummy_t, func=Act.Exp)
        # D-range scale/shift: multiplied by ASCH (schraudolph pre-scale)
        s_kvD = small.tile([KD, 1], F32, tag="s_kvD")
        t_kvD = small.tile([KD, 1], F32, tag="t_kvD")
        nc.vector.tensor_scalar(out=s_kvD, in0=s_kv[0:KD], scalar1=ASCH,
                                scalar2=None, op0=mult)
        nc.vector.tensor_scalar(out=t_kvD, in0=t_kv[0:KD], scalar1=ASCH,
                                scalar2=None, op0=mult)

        # v first: AV needs it from the first attention unit. Transposed on
        # the PE (idle during the prologue; no DMA-queue latency), drained
        # in 16-tile chunks by DVE so early k-tiles unblock AV sooner.
        with tc.high_priority():
            for b in range(B):
                vTn = vtp.tile([DV, NPAD], BF16, tag="vTn")
                nc.vector.tensor_scalar(out=vTn, in0=y_kv[32:KVP, b, :],
                                        scalar1=s_kv[32:KVP],
                                        scalar2=t_kv[32:KVP],
                                        op0=mult, op1=add)
                for (ks, ke) in ((0, 16), (16, 32), (32, 42)):
                    pst = psB.tile([128, 16, DV], BF16, tag="ps_small")
                    for j in range(ks, ke):
                        nc.tensor.transpose(pst[:, j - ks, :],
                                            vTn[:, bass.ts(j, 128)], ident32)
                    nc.vector.tensor_copy(v_aug[:, b, ks:ke, 0:DV],
                                          pst[:, 0:ke - ks, :])
                nc.gpsimd.memset(v_aug[:, b, :, DV:DV + 1], 1.0)

        # normalized k^T (17 rows: 16 ch + const row)
        # contiguous same-path runs of groups -> (tok0, tok1, is_D)
        GTOK = GRP * 128
        runs = []
        for g in range(NGRP):
            isd = g in D_SET
            if runs and runs[-1][2] == isd:
                runs[-1][1] = (g + 1) * GTOK
            else:
                runs.append([g * GTOK, (g + 1) * GTOK, isd])
        for b in range(B):
            for (a0, a1, isd) in runs:
                if b == 0:
                    nc.vector.tensor_scalar(
                        out=kT[0:KD, b, a0:a1], in0=y_kv[0:KD, b, a0:a1],
                        scalar1=s_kvD if isd else s_kv[0:KD],
                        scalar2=t_kvD if isd else t_kv[0:KD],
                        op0=mult, op1=add)
                else:
                    nc.scalar.activation(
                        out=kT[0:KD, b, a0:a1], in_=y_kv[0:KD, b, a0:a1],
                        func=Act.Identity,
                        scale=s_kvD if isd else s_kv[0:KD],
                        bias=t_kvD if isd else t_kv[0:KD])
        # const contract row (k = D-indicator, q = 16256) via DMA: engine
        # writes at partition offset 16 violate the 32-alignment rule
        for b in range(B):
            nc.sync.dma_start(out=kT[KD:KDE, b, :], in_=krowT[:])
        # zero all pad-token k rows so D-group pad psum is exactly 0
        nc.gpsimd.memset(kT[0:KD, :, N:NPAD], 0.0)

        # ------------------------- attention -------------------------
        hsT = big.tile([DV, B, NQ], BF16, tag="hsT")
        # gather bundles (per-chunk: serialized collectives each stay small
        # and finish before the next chunk's data is ready)
        BUNDLES = ((0,), (1,), (2,))
        BW = [sum(QCS[q] for q in bun) for bun in BUNDLES]
        BO = [QCO[bun[0]] for bun in BUNDLES]
        NB = len(BUNDLES)
        hs_bounces = [dram.tile([DV, B * BW[i]], BF16, tag=f"hs_bounce{i}",
                                name=f"hs_bounce{i}") for i in range(NB)]
        hs_alls = [dram.tile([H * DV, B * BW[i]], BF16, tag=f"hs_all{i}",
                             name=f"hs_all{i}") for i in range(NB)]
        for qc in range(NQC):
            w, o = QCS[qc], QCO[qc]
            avs = []
            for b in range(B):
                av_t = psB.tile([DV + 1, w], F32, tag="ps_small")
                avs.append(av_t)
            seq = []
            for gi, g in enumerate(PROC_ORDER):
                if gi < NGRP - 2:
                    seq += [(gi, g, 0), (gi, g, 1)]
            g12, g13 = PROC_ORDER[NGRP - 2], PROC_ORDER[NGRP - 1]
            seq += [(NGRP - 2, g12, 0), (NGRP - 1, g13, 0),
                    (NGRP - 2, g12, 1), (NGRP - 1, g13, 1)]
            ebs = {}
            for gi, g, b in seq:
                if g not in ebs:
                    eb = ebpool.tile([128, GRP, w], BF16, tag="eb")
                    nc.sync.dma_start(out=eb, in_=ebT[g, :, :, o:o + w])
                    ebs[g] = eb
                eb = ebs[g]
                if True:
                    qk = psA.tile([128, GRP, 512], F32, tag="qk")
                    for i in range(GRP):
                        j = g * GRP + i
                        nc.tensor.matmul(qk[:, i, 0:w],
                                         kT[:, b, bass.ts(j, 128)],
                                         qT[:, b, o:o + w],
                                         start=True, stop=True)
                    sp = spool.tile([128, GRP, w], BF16, tag="sp")
                    if g not in D_SET:
                        # A path: exact exp on ACT, then exp(bias) multiply
                        nc.scalar.activation(out=sp, in_=qk[:, :, 0:w],
                                             func=Act.Exp, scale=SCALE)
                        if g in POOL_MULT:
                            nc.gpsimd.tensor_mul(sp, sp, eb)
                        else:
                            nc.vector.tensor_mul(sp, sp, eb)
                    else:
                        # D path: schraudolph bits = psum + bias-bits -> bf16
                        eng = nc.gpsimd if g in POOL_D else nc.vector
                        eng.tensor_tensor(out=sp.bitcast(I16),
                                          in0=qk[:, :, 0:w], in1=eb, op=add)
                    for i in range(GRP):
                        j = g * GRP + i
                        nc.tensor.matmul(avs[b], v_aug[:, b, j, :],
                                         sp[:, i, :],
                                         start=(gi == 0 and i == 0),
                                         stop=(gi == NGRP - 1 and i == GRP - 1),
                                         skip_group_check=True)
            del ebs
            last_qc = qc == NQC - 1
            bun = next(i for i, bb in enumerate(BUNDLES) if qc in bb)
            for b in range(B):
                # drain: av psum -> sbuf (Pool, frees psB fast); 1/den on DVE;
                # partition-broadcast on Pool; hardswish on DVE. For the last
                # chunk read the psum directly (latency over psB recycling).
                av_sb = avs[b]
                rec = drain.tile([1, w], F32, tag="rec")
                nc.vector.reciprocal(out=rec, in_=av_sb[DV:DV + 1, :])
                recb = drain.tile([DV, w], F32, tag="recb")
                nc.gpsimd.partition_broadcast(recb, rec)
                xo = drain.tile([DV, w], BF16, tag="xo")
                nc.vector.tensor_mul(xo, av_sb[0:DV, :], recb)
                r3 = drain.tile([DV, w], BF16, tag="r3")
                nc.vector.tensor_scalar(out=r3, in0=xo, scalar1=3.0,
                                        scalar2=0.0, op0=add,
                                        op1=mybir.AluOpType.max)
                nc.vector.tensor_scalar(out=r3, in0=r3, scalar1=6.0,
                                        scalar2=1.0 / 6.0, op0=amin, op1=mult)
                nc.vector.tensor_mul(hsT[:, b, o:o + w], xo, r3)
                if qc == BUNDLES[bun][-1]:
                    # per-b bounce so b0's transfer overlaps b1's drain
                    bw, bo = BW[bun], BO[bun]
                    nc.sync.dma_start(
                        out=hs_bounces[bun].rearrange(
                            "d (b q) -> d b q", b=B)[:, b, :],
                        in_=hsT[:, b, bo:bo + bw])
            if qc == BUNDLES[bun][-1]:
                nc.gpsimd.collective_compute(
                    "AllGather", mybir.AluOpType.bypass,
                    replica_groups=[list(range(NCORES))],
                    ins=[hs_bounces[bun].opt()],
                    outs=[hs_alls[bun].opt()])

        # preload the sqrt table during the last AllGather
        nc.scalar.activation(out=dummy_t, in_=dummy_t, func=Act.Sqrt)

        # --------------------- projection (chunked) ---------------------
        y_p = big.tile([OC, B * NQ], F32, tag="yp")
        st_p = small.tile([OC, B * NQC, 6], F32, tag="st_p")
        for bun in range(NB):
            bw = BW[bun]
            hsall_sb = vtp.tile([128, 2, B * bw], BF16, tag=f"hsall{bun}",
                                name=f"hsall{bun}")
            for c in range(2):
                nc.sync.dma_start(out=hsall_sb[:, c, :],
                                  in_=hs_alls[bun][bass.ts(c, 128), :])
            for qc in BUNDLES[bun]:
                w, o = QCS[qc], QCO[qc]
                oo = o - BO[bun]
                for b in range(B):
                    ps = psB.tile([OC, w], F32, tag="ps_small")
                    for c in range(2):
                        nc.tensor.matmul(
                            ps, wp_sb[:, c, :],
                            hsall_sb[:, c, bass.ds(b * bw + oo, w)],
                            start=(c == 0), stop=(c == 1))
                    idx = b * NQC + qc
                    nc.vector.tensor_copy(
                        y_p[:, bass.ds(b * NQ + o, w)], ps)
                    nc.vector.bn_stats(
                        out=st_p[:, idx, :],
                        in_=y_p[:, bass.ds(b * NQ + o, w)])
        mv_p = small.tile([OC, 2], F32, tag="mv_p")
        nc.vector.bn_aggr(out=mv_p, in_=st_p)
        s_p, t_p = bn_scale_shift(mv_p, pgb_sb, OC, "p")
        HNQ = B * NQ // 2
        for hh in range(2):
            sl = bass.ds(hh * HNQ, HNQ)
            nc.vector.tensor_scalar(out=y_p[:, sl], in0=y_p[:, sl],
                                    scalar1=s_p, scalar2=t_p,
                                    op0=mult, op1=add)
            nc.sync.dma_start(out=yT[:, sl], in_=y_p[:, sl])

    nc.finalize()
    return nc


def _prep_inputs(x, kv_w, kv_g, kv_b, q_w, q_g, q_b, proj_w, proj_g, proj_b,
                 bias_table, bias_idxs):
    """Host-side sharding/layout prep. Returns list of 8 per-core input maps."""
    x = np.asarray(x, np.float32)
    xt = np.zeros((B, 2, 128, NPAD), np.float32)
    xTt = x.transpose(0, 2, 1)
    xt[:, :, :, :N] = xTt.reshape(B, 2, 128, N)
    xt = xt.astype(bf16)

    rank2 = np.asarray(bias_idxs)[0].reshape(ROW, COL)
    table2 = np.asarray(bias_table, np.float32)[:, rank2]  # (H, 63, 84)
    kk = np.arange(N)
    qq = np.arange(NQ)
    DRm = np.abs(kk[:, None] // COL - 2 * (qq[None, :] // COL_))
    DCm = np.abs(kk[:, None] % COL - 2 * (qq[None, :] % COL_))
    GTOK = GRP * 128
    krow = np.zeros(NPAD, np.float32)
    for g in sorted(D_SET):
        krow[g * GTOK:min((g + 1) * GTOK, N)] = 1.0
    krow = krow.astype(bf16)
    qrow = np.full(NQ, QCONST, np.float32).astype(bf16)

    in_maps = []
    for h in range(H):
        bfull = table2[h][DRm, DCm]            # (N, NQ) raw bias
        ebf = np.zeros((NPAD, NQ), np.float32)
        # A region: exp(b); pad rows stay 0 (kills pad in softmax)
        ebf[:N] = np.exp(bfull)
        # D region: bias bits add; pad rows 1.0 (bits ~0 -> tiny positive)
        for g in sorted(D_SET):
            d0, d1 = g * GTOK, min((g + 1) * GTOK, N)
            ebf[d0:d1] = EBB_SCALE * bfull[d0:d1] + EBB_SHIFT
        ebf[N:NPAD] = 1.0 if (NGRP - 1) in D_SET else 0.0
        # (NPAD, NQ) -> (NGRP, 128, GRP, NQ)
        ebl = np.ascontiguousarray(
            ebf.reshape(NGRP, GRP, 128, NQ).transpose(0, 2, 1, 3)
        ).astype(bf16)
        sl = slice(h * HKV, (h + 1) * HKV)
        slq = slice(h * KD, (h + 1) * KD)
        slo = slice(h * OC, (h + 1) * OC)
        wkv_pad = np.zeros((KVP, CIN), np.float32)
        wkv_pad[0:KD] = np.asarray(kv_w, np.float32)[sl][0:KD]
        wkv_pad[32:KVP] = np.asarray(kv_w, np.float32)[sl][KD:HKV]
        kvgb_pad = np.zeros((KVP, 4), np.float32)
        kvgb_pad[:, 0] = 1.0
        kvgb_pad[0:KD, 0] = np.asarray(kv_g, np.float32)[sl][0:KD]
        kvgb_pad[0:KD, 1] = np.asarray(kv_b, np.float32)[sl][0:KD]
        kvgb_pad[32:KVP, 0] = np.asarray(kv_g, np.float32)[sl][KD:HKV]
        kvgb_pad[32:KVP, 1] = np.asarray(kv_b, np.float32)[sl][KD:HKV]
        in_maps.append({
            "xT": xt,
            "wkvT": np.ascontiguousarray(
                wkv_pad.T.reshape(2, 128, KVP)).astype(bf16),
            "wqT": np.ascontiguousarray(
                np.asarray(q_w, np.float32)[slq].T.reshape(2, 128, KD)
            ).astype(bf16),
            "wpT": np.ascontiguousarray(
                np.asarray(proj_w, np.float32)[slo].T.reshape(2, 128, OC)
            ).astype(bf16),
            "kv_gb": np.ascontiguousarray(kvgb_pad),
            "q_gb": np.ascontiguousarray(np.stack(
                [np.asarray(q_g, np.float32)[slq],
                 np.asarray(q_b, np.float32)[slq]], axis=1)),
            "p_gb": np.ascontiguousarray(np.stack(
                [np.asarray(proj_g, np.float32)[slo],
                 np.asarray(proj_b, np.float32)[slo]], axis=1)),
            "ebT": ebl,
            "krowT": krow,
            "identT": np.eye(DV, dtype=np.float32).astype(bf16),
            "qrowT": qrow,
        })
    return in_maps


def kernel(x, kv_w, kv_g, kv_b, q_w, q_g, q_b, proj_w, proj_g, proj_b,
           bias_table, bias_idxs, _trace=False):
    global LAST_EXEC_NS
    if "nc" not in _prog_cache:
        _prog_cache["nc"] = _build_program()
    nc = _prog_cache["nc"]
    in_maps = _prep_inputs(x, kv_w, kv_g, kv_b, q_w, q_g, q_b,
                           proj_w, proj_g, proj_b, bias_table, bias_idxs)
    res = run_bass_kernel_spmd(nc, in_maps, core_ids=list(range(NCORES)),
                               trace=_trace)
    LAST_EXEC_NS = res.exec_time_ns
    yts = [np.asarray(r["yT"]) for r in res.results]
    y = np.concatenate(yts, axis=0)
    return np.ascontiguousarray(
        y.T.reshape(B, NQ, H * OC).astype(np.float32))


# revision 33
# speedup vs baseline: 1.0006x; 1.0006x over previous
"""AttentionSubsample kernel for 8 trn2 NeuronCores (head-parallel).

Sharding: 8 heads -> 8 cores; each core runs its head through attn@v and a
64-channel slice of the output projection after a per-chunk AllGather.

Engine assignment (from trace-driven iteration against the timeline model):
- All softmax exps on ACT (table exp, PSUM->bf16); splitting exp onto DVE
  (Schraudolph bit-trick, machinery still present under D_SET) measured
  slower due to cross-engine psA-slot stalls.
- exp(bias) multiplies and softmax/hardswish drain chain on DVE; PSUM
  drains of projections on ACT (Copy shares the exp table set, no reload);
  v transposed on the PE via identity matmuls (no DMA-queue latency).
- Asymmetric q-chunks (512, 512, 320) front-load ACT work so the tail
  AllGather is small and starts early; per-chunk collectives stay off each
  other's critical path; per-b bounce DMAs overlap the last drain.
- Activation-table loads hoisted via dummy Sqrt/Exp activations.
- All matmuls bf16 (fp8 QK/AV measured numerically unsafe here).
"""

import numpy as np
import ml_dtypes

import concourse.bass as bass
import concourse.mybir as mybir
import concourse.tile as tile
from concourse import bacc
from contextlib import ExitStack
from concourse.bass_utils import run_bass_kernel_spmd

BF16 = mybir.dt.bfloat16
F32 = mybir.dt.float32
I16 = mybir.dt.int16
bf16 = ml_dtypes.bfloat16

B = 2
ROW, COL = 63, 84
ROW_, COL_ = 32, 42
N = ROW * COL            # 5292 kv tokens
NQ = ROW_ * COL_         # 1344 q tokens
NPAD = 5376              # 42*128 padded kv tokens
KT = NPAD // 128         # 42 k-tiles
QC = 448                 # q chunk for projections / bn_stats
NQC = NQ // QC           # 3
# asymmetric attention q-chunks: front-load the work so the last chunk's
# AllGather (the tail) is small and starts early
QCS = (512, 512, 320)
QCO = (0, 512, 1024)
assert sum(QCS) == NQ
CIN = 256
H = 8
KD = 16
KDE = KD + 1             # +1 schraudolph-const contract row
DV = 32
HKV = KD + DV
KVP = 64                 # padded kv rows: k at 0:16, v at 32:64
OC = 64                  # per-core slice of the 512 output channels
GRP = 3                  # k-tiles per exp group
NGRP = KT // GRP         # 14
EPS = 1e-5
SCALE = KD ** -0.5
NCORES = 8

# --- engine-assignment knobs -------------------------------------------------
# group path assignment: D-groups (DVE schraudolph) interleaved among
# A-groups (ACT exp + DVE/Pool eb-mult); POOL_D run schraudolph on Pool.
D_SET = frozenset()             # DVE/Pool schraudolph groups (empty: all-ACT
                                # exp won in the sweep - no cross-engine stalls)
POOL_D = frozenset()            # subset of D_SET handled by Pool
POOL_MULT = frozenset()         # A-groups whose eb-mult runs on Pool
# process order per (qc): D-groups spread early-but-not-first so DVE's
# prologue/drain leftovers drain while ACT chews A-units (AV accumulation
# commutes, so any order is valid)
PROC_ORDER = list(range(NGRP))
assert sorted(PROC_ORDER) == list(range(NGRP))
POOL_DRAIN = False              # hardswish drain chain on Pool instead of DVE
# schraudolph constants
LOG2E = 1.4426950408889634
SCH_C = 0.0450466
ASCH = 128.0 * LOG2E * SCALE          # folded into D-range kT scale
QCONST = 16256.0                      # 128*127, exactly representable in bf16
EBB_SCALE = 128.0 * LOG2E             # bias -> bits
EBB_SHIFT = -128.0 * SCH_C            # -5.766, folded into ebb host-side

SPBUFS = 6
EBBUFS = 7

LAST_EXEC_NS = None
_prog_cache = {}


def _build_program():
    nc = bacc.Bacc(num_devices=NCORES)

    xT = nc.dram_tensor("xT", [B, 2, 128, NPAD], BF16, kind="ExternalInput")
    wkvT = nc.dram_tensor("wkvT", [2, 128, KVP], BF16, kind="ExternalInput")
    wqT = nc.dram_tensor("wqT", [2, 128, KD], BF16, kind="ExternalInput")
    wpT = nc.dram_tensor("wpT", [2, 128, OC], BF16, kind="ExternalInput")
    kv_gb = nc.dram_tensor("kv_gb", [KVP, 4], F32, kind="ExternalInput")
    q_gb = nc.dram_tensor("q_gb", [KD, 2], F32, kind="ExternalInput")
    p_gb = nc.dram_tensor("p_gb", [OC, 2], F32, kind="ExternalInput")
    ebT = nc.dram_tensor("ebT", [NGRP, 128, GRP, NQ], BF16,
                         kind="ExternalInput")
    krowT = nc.dram_tensor("krowT", [NPAD], BF16, kind="ExternalInput")
    identT = nc.dram_tensor("identT", [DV, DV], BF16, kind="ExternalInput")
    qrowT = nc.dram_tensor("qrowT", [NQ], BF16, kind="ExternalInput")
    yT = nc.dram_tensor("yT", [OC, B * NQ], F32, kind="ExternalOutput")

    with ExitStack() as ctx:
        tc = ctx.enter_context(tile.TileContext(nc))
        const = ctx.enter_context(tc.tile_pool(name="const", bufs=1))
        big = ctx.enter_context(tc.tile_pool(name="big", bufs=1))
        vtp = ctx.enter_context(tc.tile_pool(name="vtp", bufs=1))
        vtp2 = ctx.enter_context(tc.tile_pool(name="vtp2", bufs=2))
        spool = ctx.enter_context(tc.tile_pool(name="spool", bufs=SPBUFS))
        ebpool = ctx.enter_context(tc.tile_pool(name="ebpool", bufs=EBBUFS))
        small = ctx.enter_context(tc.tile_pool(name="small", bufs=4))
        drain = ctx.enter_context(tc.tile_pool(name="drain", bufs=2))
        psA = ctx.enter_context(tc.tile_pool(name="psA", bufs=2, space="PSUM"))
        psB = ctx.enter_context(tc.tile_pool(name="psB", bufs=2, space="PSUM"))
        dram = ctx.enter_context(tc.tile_pool(name="dram", bufs=4, space="DRAM"))

        mult = mybir.AluOpType.mult
        add = mybir.AluOpType.add
        amin = mybir.AluOpType.min
        Act = mybir.ActivationFunctionType

        # ------------------------- load inputs -------------------------
        xt_sb = big.tile([128, B, 2, NPAD], BF16, tag="xt")
        wkv_sb = const.tile([128, 2, KVP], BF16, tag="wkv")
        wq_sb = const.tile([128, 2, KD], BF16, tag="wq")
        wp_sb = const.tile([128, 2, OC], BF16, tag="wp")
        for c in range(2):
            nc.sync.dma_start(out=wkv_sb[:, c, :], in_=wkvT[c])
            nc.sync.dma_start(out=wq_sb[:, c, :], in_=wqT[c])
            nc.sync.dma_start(out=wp_sb[:, c, :], in_=wpT[c])
        # quarter-tensor x loads (best in the granularity sweep)
        XCH = NPAD // 4
        for t in range(4):
            for b in range(B):
                for c in range(2):
                    nc.sync.dma_start(out=xt_sb[:, b, c, bass.ts(t, XCH)],
                                      in_=xT[b, c, :, bass.ts(t, XCH)])
        kvgb_sb = const.tile([KVP, 4], F32, tag="kvgb")
        qgb_sb = const.tile([KD, 2], F32, tag="qgb")
        pgb_sb = const.tile([OC, 2], F32, tag="pgb")
        nc.sync.dma_start(out=kvgb_sb, in_=kv_gb[:, :])
        nc.sync.dma_start(out=qgb_sb, in_=q_gb[:, :])
        nc.sync.dma_start(out=pgb_sb, in_=p_gb[:, :])
        ident32 = const.tile([DV, DV], BF16, tag="ident32")
        nc.sync.dma_start(out=ident32, in_=identT[:, :])
        eps_t = const.tile([128, 1], F32, tag="eps")
        nc.vector.memset(eps_t, EPS)
        dummy_t = const.tile([1, 1], F32, tag="dummy")
        nc.vector.memset(dummy_t, 1.0)
        # force the sqrt-table load at t=0 (copy lives in every set, so the
        # prologue PSUM copies don't need another load)
        nc.scalar.activation(out=dummy_t, in_=dummy_t, func=Act.Sqrt)
        ones1_t = const.tile([1, DV], F32, tag="ones1")
        nc.vector.memset(ones1_t, 1.0)

        # ------------------------- projections -------------------------
        TCH = 448
        NT_KV = NPAD // TCH   # 12
        # projections per batch; q-proj reads the subsample via a strided
        # view of xt (no separate xs DMA); stats interleaved
        QPC = 336
        y_q = big.tile([KD, B, NQ], BF16, tag="yq")
        st_q = small.tile([KD, 2 * 4, 6], F32, tag="st_q")
        y_kv = big.tile([KVP, B, NPAD], BF16, tag="ykv")
        st_kv = small.tile([KVP, 2 * NT_KV, 6], F32, tag="st_kv")
        for t4 in range(4):
            for b in range(B):
                for t in range(3 * t4, 3 * t4 + 3):
                    ps = psB.tile([KVP, TCH], F32, tag="ps_small")
                    for c in range(2):
                        nc.tensor.matmul(ps, wkv_sb[:, c, :],
                                         xt_sb[:, b, c, bass.ts(t, TCH)],
                                         start=(c == 0), stop=(c == 1))
                    nc.scalar.copy(out=y_kv[:, b, bass.ts(t, TCH)], in_=ps)
                    if t > 0:
                        i = t - 1
                        nc.vector.bn_stats(out=st_kv[:, b * NT_KV + i, :],
                                           in_=y_kv[:, b, bass.ds(i * 441, 441)])
            for b in range(B):
                t = t4
                ps = psB.tile([KD, QPC], F32, tag="ps_small")
                for c in range(2):
                    xsv = xt_sb[:, b, c, 0:N].rearrange(
                        "p (r c2) -> p r c2",
                        r=ROW)[:, 16 * t:min(16 * (t + 1), ROW):2, ::2]
                    nc.tensor.matmul(ps, wq_sb[:, c, :], xsv,
                                     start=(c == 0), stop=(c == 1))
                nc.scalar.copy(out=y_q[:, b, bass.ts(t, QPC)], in_=ps)
                nc.vector.bn_stats(out=st_q[:, b * 4 + t, :],
                                   in_=y_q[:, b, bass.ts(t, QPC)])
        for b in range(B):
            nc.vector.bn_stats(out=st_kv[:, b * NT_KV + NT_KV - 1, :],
                               in_=y_kv[:, b, bass.ds(11 * 441, 441)])

        # ------------------------- batch norms -------------------------
        def bn_scale_shift(mv, gb, P, name):
            s = small.tile([P, 1], F32, tag=f"s_{name}")
            t = small.tile([P, 1], F32, tag=f"t_{name}")
            nc.scalar.activation(out=s, in_=mv[:, 1:2], func=Act.Sqrt,
                                 bias=eps_t[0:P])
            nc.vector.reciprocal(out=s, in_=s)
            nc.vector.tensor_mul(s, s, gb[:, 0:1])
            nc.vector.tensor_mul(t, mv[:, 0:1], s)
            nc.vector.tensor_scalar(out=t, in0=t, scalar1=-1.0, scalar2=None,
                                    op0=mult)
            nc.vector.tensor_add(t, t, gb[:, 1:2])
            return s, t

        mv_q = small.tile([KD, 2], F32, tag="mv_q")
        nc.vector.bn_aggr(out=mv_q, in_=st_q)
        s_q, t_q = bn_scale_shift(mv_q, qgb_sb, KD, "q")

        kT = big.tile([KDE, B, NPAD], BF16, tag="kT")
        qT = big.tile([KDE, B, NQ], BF16, tag="qT")
        v_aug = big.tile([128, B, KT, DV + 1], BF16, tag="vaug")
        # qT norm early (only needs q stats); b1 on ACT
        nc.vector.tensor_scalar(out=qT[0:KD, 0, :], in0=y_q[:, 0, :],
                                scalar1=s_q, scalar2=t_q,
                                op0=mult, op1=add)
        nc.scalar.activation(out=qT[0:KD, 1, :], in_=y_q[:, 1, :],
                             func=Act.Identity, scale=s_q, bias=t_q)
        for b in range(B):
            nc.sync.dma_start(out=qT[KD:KDE, b, :], in_=qrowT[:])

        mv_kv = small.tile([KVP, 2], F32, tag="mv_kv")
        nc.vector.bn_aggr(out=mv_kv, in_=st_kv)
        s_kv, t_kv = bn_scale_shift(mv_kv, kvgb_sb, KVP, "kv")
        nc.scalar.activation(out=dummy_t, in_=dummy_t, func=Act.Exp)
        # D-range scale/shift: multiplied by ASCH (schraudolph pre-scale)
        s_kvD = small.tile([KD, 1], F32, tag="s_kvD")
        t_kvD = small.tile([KD, 1], F32, tag="t_kvD")
        nc.vector.tensor_scalar(out=s_kvD, in0=s_kv[0:KD], scalar1=ASCH,
                                scalar2=None, op0=mult)
        nc.vector.tensor_scalar(out=t_kvD, in0=t_kv[0:KD], scalar1=ASCH,
                                scalar2=None, op0=mult)

        # v first: AV needs it from the first attention unit. Transposed on
        # the PE (idle during the prologue; no DMA-queue latency), drained
        # in 16-tile chunks by DVE so early k-tiles unblock AV sooner.
        with tc.high_priority():
            for b in range(B):
                vTn = vtp.tile([DV, NPAD], BF16, tag="vTn")
                nc.vector.tensor_scalar(out=vTn, in0=y_kv[32:KVP, b, :],
                                        scalar1=s_kv[32:KVP],
                                        scalar2=t_kv[32:KVP],
                                        op0=mult, op1=add)
                for (ks, ke) in ((0, 16), (16, 32), (32, 42)):
                    pst = psB.tile([128, 16, DV], BF16, tag="ps_small")
                    for j in range(ks, ke):
                        nc.tensor.transpose(pst[:, j - ks, :],
                                            vTn[:, bass.ts(j, 128)], ident32)
                    nc.vector.tensor_copy(v_aug[:, b, ks:ke, 0:DV],
                                          pst[:, 0:ke - ks, :])
                nc.gpsimd.memset(v_aug[:, b, :, DV:DV + 1], 1.0)

        # normalized k^T (17 rows: 16 ch + const row)
        # contiguous same-path runs of groups -> (tok0, tok1, is_D)
        GTOK = GRP * 128
        runs = []
        for g in range(NGRP):
            isd = g in D_SET
            if runs and runs[-1][2] == isd:
                runs[-1][1] = (g + 1) * GTOK
            else:
                runs.append([g * GTOK, (g + 1) * GTOK, isd])
        for b in range(B):
            for (a0, a1, isd) in runs:
                if b == 0:
                    nc.vector.tensor_scalar(
                        out=kT[0:KD, b, a0:a1], in0=y_kv[0:KD, b, a0:a1],
                        scalar1=s_kvD if isd else s_kv[0:KD],
                        scalar2=t_kvD if isd else t_kv[0:KD],
                        op0=mult, op1=add)
                else:
                    nc.scalar.activation(
                        out=kT[0:KD, b, a0:a1], in_=y_kv[0:KD, b, a0:a1],
                        func=Act.Identity,
                        scale=s_kvD if isd else s_kv[0:KD],
                        bias=t_kvD if isd else t_kv[0:KD])
        # const contract row (k = D-indicator, q = 16256) via DMA: engine
        # writes at partition offset 16 violate the 32-alignment rule
        for b in range(B):
            nc.sync.dma_start(out=kT[KD:KDE, b, :], in_=krowT[:])
        # zero all pad-token k rows so D-group pad psum is exactly 0
        nc.gpsimd.memset(kT[0:KD, :, N:NPAD], 0.0)

        # ------------------------- attention -------------------------
        hsT = big.tile([DV, B, NQ], BF16, tag="hsT")
        # gather bundles (per-chunk: serialized collectives each stay small
        # and finish before the next chunk's data is ready)
        BUNDLES = ((0,), (1,), (2,))
        BW = [sum(QCS[q] for q in bun) for bun in BUNDLES]
        BO = [QCO[bun[0]] for bun in BUNDLES]
        NB = len(BUNDLES)
        hs_bounces = [dram.tile([DV, B * BW[i]], BF16, tag=f"hs_bounce{i}",
                                name=f"hs_bounce{i}") for i in range(NB)]
        hs_alls = [dram.tile([H * DV, B * BW[i]], BF16, tag=f"hs_all{i}",
                             name=f"hs_all{i}") for i in range(NB)]
        for qc in range(NQC):
            w, o = QCS[qc], QCO[qc]
            avs = []
            for b in range(B):
                av_t = psB.tile([DV + 1, w], F32, tag="ps_small")
                avs.append(av_t)
            seq = []
            for gi, g in enumerate(PROC_ORDER):
                if gi < NGRP - 2:
                    seq += [(gi, g, 0), (gi, g, 1)]
            g12, g13 = PROC_ORDER[NGRP - 2], PROC_ORDER[NGRP - 1]
            seq += [(NGRP - 2, g12, 0), (NGRP - 1, g13, 0),
                    (NGRP - 2, g12, 1), (NGRP - 1, g13, 1)]
            ebs = {}
            for gi, g, b in seq:
                if g not in ebs:
                    eb = ebpool.tile([128, GRP, w], BF16, tag="eb")
                    nc.sync.dma_start(out=eb, in_=ebT[g, :, :, o:o + w])
                    ebs[g] = eb
                eb = ebs[g]
                if True:
                    qk = psA.tile([128, GRP, 512], F32, tag="qk")
                    for i in range(GRP):
                        j = g * GRP + i
                        nc.tensor.matmul(qk[:, i, 0:w],
                                         kT[:, b, bass.ts(j, 128)],
                                         qT[:, b, o:o + w],
                                         start=True, stop=True)
                    sp = spool.tile([128, GRP, w], BF16, tag="sp")
                    if g not in D_SET:
                        # A path: exact exp on ACT, then exp(bias) multiply
                        nc.scalar.activation(out=sp, in_=qk[:, :, 0:w],
                                             func=Act.Exp, scale=SCALE)
                        if g in POOL_MULT:
                            nc.gpsimd.tensor_mul(sp, sp, eb)
                        else:
                            nc.vector.tensor_mul(sp, sp, eb)
                    else:
                        # D path: schraudolph bits = psum + bias-bits -> bf16
                        eng = nc.gpsimd if g in POOL_D else nc.vector
                        eng.tensor_tensor(out=sp.bitcast(I16),
                                          in0=qk[:, :, 0:w], in1=eb, op=add)
                    for i in range(GRP):
                        j = g * GRP + i
                        nc.tensor.matmul(avs[b], v_aug[:, b, j, :],
                                         sp[:, i, :],
                                         start=(gi == 0 and i == 0),
                                         stop=(gi == NGRP - 1 and i == GRP - 1),
                                         skip_group_check=True)
            del ebs
            last_qc = qc == NQC - 1
            bun = next(i for i, bb in enumerate(BUNDLES) if qc in bb)
            for b in range(B):
                # drain: av psum -> sbuf (Pool, frees psB fast); 1/den on DVE;
                # partition-broadcast on Pool; hardswish on DVE. For the last
                # chunk read the psum directly (latency over psB recycling).
                av_sb = avs[b]
                rec = drain.tile([1, w], F32, tag="rec")
                nc.vector.reciprocal(out=rec, in_=av_sb[DV:DV + 1, :])
                recb = drain.tile([DV, w], F32, tag="recb")
                nc.gpsimd.partition_broadcast(recb, rec)
                xo = drain.tile([DV, w], BF16, tag="xo")
                nc.vector.tensor_mul(xo, av_sb[0:DV, :], recb)
                r3 = drain.tile([DV, w], BF16, tag="r3")
                nc.vector.tensor_scalar(out=r3, in0=xo, scalar1=3.0,
                                        scalar2=0.0, op0=add,
                                        op1=mybir.AluOpType.max)
                nc.vector.tensor_scalar(out=r3, in0=r3, scalar1=6.0,
                                        scalar2=1.0 / 6.0, op0=amin, op1=mult)
                nc.vector.tensor_mul(hsT[:, b, o:o + w], xo, r3)
                if qc == BUNDLES[bun][-1]:
                    # per-b bounce so b0's transfer overlaps b1's drain
                    bw, bo = BW[bun], BO[bun]
                    nc.sync.dma_start(
                        out=hs_bounces[bun].rearrange(
                            "d (b q) -> d b q", b=B)[:, b, :],
                        in_=hsT[:, b, bo:bo + bw])
            if qc == BUNDLES[bun][-1]:
                nc.gpsimd.collective_compute(
                    "AllGather", mybir.AluOpType.bypass,
                    replica_groups=[list(range(NCORES))],
                    ins=[hs_bounces[bun].opt()],
                    outs=[hs_alls[bun].opt()])

        # preload the sqrt table during the last AllGather
        nc.scalar.activation(out=dummy_t, in_=dummy_t, func=Act.Sqrt)

        # --------------------- projection (chunked) ---------------------
        y_p = big.tile([OC, B * NQ], F32, tag="yp")
        st_p = small.tile([OC, B * NQC, 6], F32, tag="st_p")
        tail_ps = []
        for bun in range(NB):
            bw = BW[bun]
            hsall_sb = vtp.tile([128, 2, B * bw], BF16, tag=f"hsall{bun}",
                                name=f"hsall{bun}")
            for c in range(2):
                nc.sync.dma_start(out=hsall_sb[:, c, :],
                                  in_=hs_alls[bun][bass.ts(c, 128), :])
            last_bun = bun == NB - 1
            for qc in BUNDLES[bun]:
                w, o = QCS[qc], QCO[qc]
                oo = o - BO[bun]
                for b in range(B):
                    ps = psB.tile([OC, w], F32, tag="ps_small")
                    for c in range(2):
                        nc.tensor.matmul(
                            ps, wp_sb[:, c, :],
                            hsall_sb[:, c, bass.ds(b * bw + oo, w)],
                            start=(c == 0), stop=(c == 1))
                    idx = b * NQC + qc
                    if last_bun:
                        # tail chunk: stats straight off PSUM; the BN scale
                        # is fused into the psum drain below (after aggr)
                        nc.vector.bn_stats(out=st_p[:, idx, :], in_=ps)
                        tail_ps.append((ps, b, o, w))
                    else:
                        nc.vector.tensor_copy(
                            y_p[:, bass.ds(b * NQ + o, w)], ps)
                        nc.vector.bn_stats(
                            out=st_p[:, idx, :],
                            in_=y_p[:, bass.ds(b * NQ + o, w)])
        mv_p = small.tile([OC, 2], F32, tag="mv_p")
        nc.vector.bn_aggr(out=mv_p, in_=st_p)
        s_p, t_p = bn_scale_shift(mv_p, pgb_sb, OC, "p")
        # tail chunk: scaled drain + its own output DMA
        for (ps, b, o, w) in tail_ps:
            nc.vector.tensor_scalar(out=y_p[:, bass.ds(b * NQ + o, w)],
                                    in0=ps, scalar1=s_p, scalar2=t_p,
                                    op0=mult, op1=add)
            nc.sync.dma_start(out=yT[:, bass.ds(b * NQ + o, w)],
                              in_=y_p[:, bass.ds(b * NQ + o, w)])
        # earlier chunks: scale + DMA per batch-half
        for b in range(B):
            o2, w2 = QCO[0], QCS[0] + QCS[1]
            sl = bass.ds(b * NQ + o2, w2)
            nc.vector.tensor_scalar(out=y_p[:, sl], in0=y_p[:, sl],
                                    scalar1=s_p, scalar2=t_p,
                                    op0=mult, op1=add)
            nc.sync.dma_start(out=yT[:, sl], in_=y_p[:, sl])

    nc.finalize()
    return nc


def _prep_inputs(x, kv_w, kv_g, kv_b, q_w, q_g, q_b, proj_w, proj_g, proj_b,
                 bias_table, bias_idxs):
    """Host-side sharding/layout prep. Returns list of 8 per-core input maps."""
    x = np.asarray(x, np.float32)
    xt = np.zeros((B, 2, 128, NPAD), np.float32)
    xTt = x.transpose(0, 2, 1)
    xt[:, :, :, :N] = xTt.reshape(B, 2, 128, N)
    xt = xt.astype(bf16)

    rank2 = np.asarray(bias_idxs)[0].reshape(ROW, COL)
    table2 = np.asarray(bias_table, np.float32)[:, rank2]  # (H, 63, 84)
    kk = np.arange(N)
    qq = np.arange(NQ)
    DRm = np.abs(kk[:, None] // COL - 2 * (qq[None, :] // COL_))
    DCm = np.abs(kk[:, None] % COL - 2 * (qq[None, :] % COL_))
    GTOK = GRP * 128
    krow = np.zeros(NPAD, np.float32)
    for g in sorted(D_SET):
        krow[g * GTOK:min((g + 1) * GTOK, N)] = 1.0
    krow = krow.astype(bf16)
    qrow = np.full(NQ, QCONST, np.float32).astype(bf16)

    in_maps = []
    for h in range(H):
        bfull = table2[h][DRm, DCm]            # (N, NQ) raw bias
        ebf = np.zeros((NPAD, NQ), np.float32)
        # A region: exp(b); pad rows stay 0 (kills pad in softmax)
        ebf[:N] = np.exp(bfull)
        # D region: bias bits add; pad rows 1.0 (bits ~0 -> tiny positive)
        for g in sorted(D_SET):
            d0, d1 = g * GTOK, min((g + 1) * GTOK, N)
            ebf[d0:d1] = EBB_SCALE * bfull[d0:d1] + EBB_SHIFT
        ebf[N:NPAD] = 1.0 if (NGRP - 1) in D_SET else 0.0
        # (NPAD, NQ) -> (NGRP, 128, GRP, NQ)
        ebl = np.ascontiguousarray(
            ebf.reshape(NGRP, GRP, 128, NQ).transpose(0, 2, 1, 3)
        ).astype(bf16)
        sl = slice(h * HKV, (h + 1) * HKV)
        slq = slice(h * KD, (h + 1) * KD)
        slo = slice(h * OC, (h + 1) * OC)
        wkv_pad = np.zeros((KVP, CIN), np.float32)
        wkv_pad[0:KD] = np.asarray(kv_w, np.float32)[sl][0:KD]
        wkv_pad[32:KVP] = np.asarray(kv_w, np.float32)[sl][KD:HKV]
        kvgb_pad = np.zeros((KVP, 4), np.float32)
        kvgb_pad[:, 0] = 1.0
        kvgb_pad[0:KD, 0] = np.asarray(kv_g, np.float32)[sl][0:KD]
        kvgb_pad[0:KD, 1] = np.asarray(kv_b, np.float32)[sl][0:KD]
        kvgb_pad[32:KVP, 0] = np.asarray(kv_g, np.float32)[sl][KD:HKV]
        kvgb_pad[32:KVP, 1] = np.asarray(kv_b, np.float32)[sl][KD:HKV]
        in_maps.append({
            "xT": xt,
            "wkvT": np.ascontiguousarray(
                wkv_pad.T.reshape(2, 128, KVP)).astype(bf16),
            "wqT": np.ascontiguousarray(
                np.asarray(q_w, np.float32)[slq].T.reshape(2, 128, KD)
            ).astype(bf16),
            "wpT": np.ascontiguousarray(
                np.asarray(proj_w, np.float32)[slo].T.reshape(2, 128, OC)
            ).astype(bf16),
            "kv_gb": np.ascontiguousarray(kvgb_pad),
            "q_gb": np.ascontiguousarray(np.stack(
                [np.asarray(q_g, np.float32)[slq],
                 np.asarray(q_b, np.float32)[slq]], axis=1)),
            "p_gb": np.ascontiguousarray(np.stack(
                [np.asarray(proj_g, np.float32)[slo],
                 np.asarray(proj_b, np.float32)[slo]], axis=1)),
            "ebT": ebl,
            "krowT": krow,
            "identT": np.eye(DV, dtype=np.float32).astype(bf16),
            "qrowT": qrow,
        })
    return in_maps


def kernel(x, kv_w, kv_g, kv_b, q_w, q_g, q_b, proj_w, proj_g, proj_b,
           bias_table, bias_idxs, _trace=False):
    global LAST_EXEC_NS
    if "nc" not in _prog_cache:
        _prog_cache["nc"] = _build_program()
    nc = _prog_cache["nc"]
    in_maps = _prep_inputs(x, kv_w, kv_g, kv_b, q_w, q_g, q_b,
                           proj_w, proj_g, proj_b, bias_table, bias_idxs)
    res = run_bass_kernel_spmd(nc, in_maps, core_ids=list(range(NCORES)),
                               trace=_trace)
    LAST_EXEC_NS = res.exec_time_ns
    yts = [np.asarray(r["yT"]) for r in res.results]
    y = np.concatenate(yts, axis=0)
    return np.ascontiguousarray(
        y.T.reshape(B, NQ, H * OC).astype(np.float32))


# revision 35
# speedup vs baseline: 1.0009x; 1.0003x over previous
"""AttentionSubsample kernel for 8 trn2 NeuronCores (head-parallel).

Sharding: 8 heads -> 8 cores; each core runs its head through attn@v and a
64-channel slice of the output projection after a per-chunk AllGather.

Engine assignment (from trace-driven iteration against the timeline model):
- All softmax exps on ACT (table exp, PSUM->bf16); splitting exp onto DVE
  (Schraudolph bit-trick, machinery still present under D_SET) measured
  slower due to cross-engine psA-slot stalls.
- exp(bias) multiplies and softmax/hardswish drain chain on DVE; PSUM
  drains of projections on ACT (Copy shares the exp table set, no reload);
  v transposed on the PE via identity matmuls (no DMA-queue latency).
- Asymmetric q-chunks (512, 512, 320) front-load ACT work so the tail
  AllGather is small and starts early; per-chunk collectives stay off each
  other's critical path; per-b bounce DMAs overlap the last drain.
- Activation-table loads hoisted via dummy Sqrt/Exp activations.
- All matmuls bf16 (fp8 QK/AV measured numerically unsafe here).
"""

import numpy as np
import ml_dtypes

import concourse.bass as bass
import concourse.mybir as mybir
import concourse.tile as tile
from concourse import bacc
from contextlib import ExitStack
from concourse.bass_utils import run_bass_kernel_spmd

BF16 = mybir.dt.bfloat16
F32 = mybir.dt.float32
I16 = mybir.dt.int16
bf16 = ml_dtypes.bfloat16

B = 2
ROW, COL = 63, 84
ROW_, COL_ = 32, 42
N = ROW * COL            # 5292 kv tokens
NQ = ROW_ * COL_         # 1344 q tokens
NPAD = 5376              # 42*128 padded kv tokens
KT = NPAD // 128         # 42 k-tiles
QC = 448                 # q chunk for projections / bn_stats
NQC = NQ // QC           # 3
# asymmetric attention q-chunks: front-load the work so the last chunk's
# AllGather (the tail) is small and starts early
QCS = (512, 512, 320)
QCO = (0, 512, 1024)
assert sum(QCS) == NQ
CIN = 256
H = 8
KD = 16
KDE = KD + 1             # +1 schraudolph-const contract row
DV = 32
HKV = KD + DV
KVP = 64                 # padded kv rows: k at 0:16, v at 32:64
OC = 64                  # per-core slice of the 512 output channels
GRP = 3                  # k-tiles per exp group
NGRP = KT // GRP         # 14
EPS = 1e-5
SCALE = KD ** -0.5
NCORES = 8

# --- engine-assignment knobs -------------------------------------------------
# group path assignment: D-groups (DVE schraudolph) interleaved among
# A-groups (ACT exp + DVE/Pool eb-mult); POOL_D run schraudolph on Pool.
D_SET = frozenset()             # DVE/Pool schraudolph groups (empty: all-ACT
                                # exp won in the sweep - no cross-engine stalls)
POOL_D = frozenset()            # subset of D_SET handled by Pool
POOL_MULT = frozenset()         # A-groups whose eb-mult runs on Pool
# process order per (qc): D-groups spread early-but-not-first so DVE's
# prologue/drain leftovers drain while ACT chews A-units (AV accumulation
# commutes, so any order is valid)
PROC_ORDER = list(range(NGRP))
assert sorted(PROC_ORDER) == list(range(NGRP))
POOL_DRAIN = False              # hardswish drain chain on Pool instead of DVE
# schraudolph constants
LOG2E = 1.4426950408889634
SCH_C = 0.0450466
ASCH = 128.0 * LOG2E * SCALE          # folded into D-range kT scale
QCONST = 16256.0                      # 128*127, exactly representable in bf16
EBB_SCALE = 128.0 * LOG2E             # bias -> bits
EBB_SHIFT = -128.0 * SCH_C            # -5.766, folded into ebb host-side

SPBUFS = 6
EBBUFS = 7

LAST_EXEC_NS = None
_prog_cache = {}


def _build_program():
    nc = bacc.Bacc(num_devices=NCORES)

    xT = nc.dram_tensor("xT", [B, 2, 128, NPAD], BF16, kind="ExternalInput")
    wkvT = nc.dram_tensor("wkvT", [2, 128, KVP], BF16, kind="ExternalInput")
    wqT = nc.dram_tensor("wqT", [2, 128, KD], BF16, kind="ExternalInput")
    wpT = nc.dram_tensor("wpT", [2, 128, OC], BF16, kind="ExternalInput")
    kv_gb = nc.dram_tensor("kv_gb", [KVP, 4], F32, kind="ExternalInput")
    q_gb = nc.dram_tensor("q_gb", [KD, 2], F32, kind="ExternalInput")
    p_gb = nc.dram_tensor("p_gb", [OC, 2], F32, kind="ExternalInput")
    ebT = nc.dram_tensor("ebT", [NGRP, 128, GRP, NQ], BF16,
                         kind="ExternalInput")
    krowT = nc.dram_tensor("krowT", [NPAD], BF16, kind="ExternalInput")
    identT = nc.dram_tensor("identT", [DV, DV], BF16, kind="ExternalInput")
    qrowT = nc.dram_tensor("qrowT", [NQ], BF16, kind="ExternalInput")
    yT = nc.dram_tensor("yT", [OC, B * NQ], F32, kind="ExternalOutput")

    with ExitStack() as ctx:
        tc = ctx.enter_context(tile.TileContext(nc))
        const = ctx.enter_context(tc.tile_pool(name="const", bufs=1))
        big = ctx.enter_context(tc.tile_pool(name="big", bufs=1))
        vtp = ctx.enter_context(tc.tile_pool(name="vtp", bufs=1))
        vtp2 = ctx.enter_context(tc.tile_pool(name="vtp2", bufs=2))
        spool = ctx.enter_context(tc.tile_pool(name="spool", bufs=SPBUFS))
        ebpool = ctx.enter_context(tc.tile_pool(name="ebpool", bufs=EBBUFS))
        small = ctx.enter_context(tc.tile_pool(name="small", bufs=4))
        drain = ctx.enter_context(tc.tile_pool(name="drain", bufs=3))
        psA = ctx.enter_context(tc.tile_pool(name="psA", bufs=2, space="PSUM"))
        psB = ctx.enter_context(tc.tile_pool(name="psB", bufs=2, space="PSUM"))
        dram = ctx.enter_context(tc.tile_pool(name="dram", bufs=4, space="DRAM"))

        mult = mybir.AluOpType.mult
        add = mybir.AluOpType.add
        amin = mybir.AluOpType.min
        Act = mybir.ActivationFunctionType

        # ------------------------- load inputs -------------------------
        xt_sb = big.tile([128, B, 2, NPAD], BF16, tag="xt")
        wkv_sb = const.tile([128, 2, KVP], BF16, tag="wkv")
        wq_sb = const.tile([128, 2, KD], BF16, tag="wq")
        wp_sb = const.tile([128, 2, OC], BF16, tag="wp")
        for c in range(2):
            nc.sync.dma_start(out=wkv_sb[:, c, :], in_=wkvT[c])
            nc.sync.dma_start(out=wq_sb[:, c, :], in_=wqT[c])
            nc.sync.dma_start(out=wp_sb[:, c, :], in_=wpT[c])
        # quarter-tensor x loads (best in the granularity sweep)
        XCH = NPAD // 4
        for t in range(4):
            for b in range(B):
                for c in range(2):
                    nc.sync.dma_start(out=xt_sb[:, b, c, bass.ts(t, XCH)],
                                      in_=xT[b, c, :, bass.ts(t, XCH)])
        kvgb_sb = const.tile([KVP, 4], F32, tag="kvgb")
        qgb_sb = const.tile([KD, 2], F32, tag="qgb")
        pgb_sb = const.tile([OC, 2], F32, tag="pgb")
        nc.sync.dma_start(out=kvgb_sb, in_=kv_gb[:, :])
        nc.sync.dma_start(out=qgb_sb, in_=q_gb[:, :])
        nc.sync.dma_start(out=pgb_sb, in_=p_gb[:, :])
        ident32 = const.tile([DV, DV], BF16, tag="ident32")
        nc.sync.dma_start(out=ident32, in_=identT[:, :])
        eps_t = const.tile([128, 1], F32, tag="eps")
        nc.vector.memset(eps_t, EPS)
        dummy_t = const.tile([1, 1], F32, tag="dummy")
        nc.vector.memset(dummy_t, 1.0)
        # force the sqrt-table load at t=0 (copy lives in every set, so the
        # prologue PSUM copies don't need another load)
        nc.scalar.activation(out=dummy_t, in_=dummy_t, func=Act.Sqrt)
        ones1_t = const.tile([1, DV], F32, tag="ones1")
        nc.vector.memset(ones1_t, 1.0)

        # ------------------------- projections -------------------------
        TCH = 448
        NT_KV = NPAD // TCH   # 12
        # projections per batch; q-proj reads the subsample via a strided
        # view of xt (no separate xs DMA); stats interleaved
        QPC = 336
        y_q = big.tile([KD, B, NQ], BF16, tag="yq")
        st_q = small.tile([KD, 2 * 4, 6], F32, tag="st_q")
        y_kv = big.tile([KVP, B, NPAD], BF16, tag="ykv")
        st_kv = small.tile([KVP, 2 * NT_KV, 6], F32, tag="st_kv")
        for t4 in range(4):
            for b in range(B):
                for t in range(3 * t4, 3 * t4 + 3):
                    ps = psB.tile([KVP, TCH], F32, tag="ps_small")
                    for c in range(2):
                        nc.tensor.matmul(ps, wkv_sb[:, c, :],
                                         xt_sb[:, b, c, bass.ts(t, TCH)],
                                         start=(c == 0), stop=(c == 1))
                    nc.scalar.copy(out=y_kv[:, b, bass.ts(t, TCH)], in_=ps)
                    if t > 0:
                        i = t - 1
                        nc.vector.bn_stats(out=st_kv[:, b * NT_KV + i, :],
                                           in_=y_kv[:, b, bass.ds(i * 441, 441)])
            for b in range(B):
                t = t4
                ps = psB.tile([KD, QPC], F32, tag="ps_small")
                for c in range(2):
                    xsv = xt_sb[:, b, c, 0:N].rearrange(
                        "p (r c2) -> p r c2",
                        r=ROW)[:, 16 * t:min(16 * (t + 1), ROW):2, ::2]
                    nc.tensor.matmul(ps, wq_sb[:, c, :], xsv,
                                     start=(c == 0), stop=(c == 1))
                nc.scalar.copy(out=y_q[:, b, bass.ts(t, QPC)], in_=ps)
                nc.vector.bn_stats(out=st_q[:, b * 4 + t, :],
                                   in_=y_q[:, b, bass.ts(t, QPC)])
        for b in range(B):
            nc.vector.bn_stats(out=st_kv[:, b * NT_KV + NT_KV - 1, :],
                               in_=y_kv[:, b, bass.ds(11 * 441, 441)])

        # ------------------------- batch norms -------------------------
        def bn_scale_shift(mv, gb, P, name):
            s = small.tile([P, 1], F32, tag=f"s_{name}")
            t = small.tile([P, 1], F32, tag=f"t_{name}")
            nc.scalar.activation(out=s, in_=mv[:, 1:2], func=Act.Sqrt,
                                 bias=eps_t[0:P])
            nc.vector.reciprocal(out=s, in_=s)
            nc.vector.tensor_mul(s, s, gb[:, 0:1])
            nc.vector.tensor_mul(t, mv[:, 0:1], s)
            nc.vector.tensor_scalar(out=t, in0=t, scalar1=-1.0, scalar2=None,
                                    op0=mult)
            nc.vector.tensor_add(t, t, gb[:, 1:2])
            return s, t

        mv_q = small.tile([KD, 2], F32, tag="mv_q")
        nc.vector.bn_aggr(out=mv_q, in_=st_q)
        s_q, t_q = bn_scale_shift(mv_q, qgb_sb, KD, "q")

        kT = big.tile([KDE, B, NPAD], BF16, tag="kT")
        qT = big.tile([KDE, B, NQ], BF16, tag="qT")
        v_aug = big.tile([128, B, KT, DV + 1], BF16, tag="vaug")
        # qT norm early (only needs q stats); b1 on ACT
        nc.vector.tensor_scalar(out=qT[0:KD, 0, :], in0=y_q[:, 0, :],
                                scalar1=s_q, scalar2=t_q,
                                op0=mult, op1=add)
        nc.scalar.activation(out=qT[0:KD, 1, :], in_=y_q[:, 1, :],
                             func=Act.Identity, scale=s_q, bias=t_q)
        for b in range(B):
            nc.sync.dma_start(out=qT[KD:KDE, b, :], in_=qrowT[:])

        mv_kv = small.tile([KVP, 2], F32, tag="mv_kv")
        nc.vector.bn_aggr(out=mv_kv, in_=st_kv)
        s_kv, t_kv = bn_scale_shift(mv_kv, kvgb_sb, KVP, "kv")
        nc.scalar.activation(out=dummy_t, in_=dummy_t, func=Act.Exp)
        # D-range scale/shift: multiplied by ASCH (schraudolph pre-scale)
        s_kvD = small.tile([KD, 1], F32, tag="s_kvD")
        t_kvD = small.tile([KD, 1], F32, tag="t_kvD")
        nc.vector.tensor_scalar(out=s_kvD, in0=s_kv[0:KD], scalar1=ASCH,
                                scalar2=None, op0=mult)
        nc.vector.tensor_scalar(out=t_kvD, in0=t_kv[0:KD], scalar1=ASCH,
                                scalar2=None, op0=mult)

        # v first: AV needs it from the first attention unit. Transposed on
        # the PE (idle during the prologue; no DMA-queue latency), drained
        # in 16-tile chunks by DVE so early k-tiles unblock AV sooner.
        with tc.high_priority():
            for b in range(B):
                vTn = vtp.tile([DV, NPAD], BF16, tag="vTn")
                nc.vector.tensor_scalar(out=vTn, in0=y_kv[32:KVP, b, :],
                                        scalar1=s_kv[32:KVP],
                                        scalar2=t_kv[32:KVP],
                                        op0=mult, op1=add)
                for (ks, ke) in ((0, 16), (16, 32), (32, 42)):
                    pst = psB.tile([128, 16, DV], BF16, tag="ps_small")
                    for j in range(ks, ke):
                        nc.tensor.transpose(pst[:, j - ks, :],
                                            vTn[:, bass.ts(j, 128)], ident32)
                    nc.vector.tensor_copy(v_aug[:, b, ks:ke, 0:DV],
                                          pst[:, 0:ke - ks, :])
                nc.gpsimd.memset(v_aug[:, b, :, DV:DV + 1], 1.0)

        # normalized k^T (17 rows: 16 ch + const row)
        # contiguous same-path runs of groups -> (tok0, tok1, is_D)
        GTOK = GRP * 128
        runs = []
        for g in range(NGRP):
            isd = g in D_SET
            if runs and runs[-1][2] == isd:
                runs[-1][1] = (g + 1) * GTOK
            else:
                runs.append([g * GTOK, (g + 1) * GTOK, isd])
        for b in range(B):
            for (a0, a1, isd) in runs:
                if b == 0:
                    nc.vector.tensor_scalar(
                        out=kT[0:KD, b, a0:a1], in0=y_kv[0:KD, b, a0:a1],
                        scalar1=s_kvD if isd else s_kv[0:KD],
                        scalar2=t_kvD if isd else t_kv[0:KD],
                        op0=mult, op1=add)
                else:
                    nc.scalar.activation(
                        out=kT[0:KD, b, a0:a1], in_=y_kv[0:KD, b, a0:a1],
                        func=Act.Identity,
                        scale=s_kvD if isd else s_kv[0:KD],
                        bias=t_kvD if isd else t_kv[0:KD])
        # const contract row (k = D-indicator, q = 16256) via DMA: engine
        # writes at partition offset 16 violate the 32-alignment rule
        for b in range(B):
            nc.sync.dma_start(out=kT[KD:KDE, b, :], in_=krowT[:])
        # zero all pad-token k rows so D-group pad psum is exactly 0
        nc.gpsimd.memset(kT[0:KD, :, N:NPAD], 0.0)

        # ------------------------- attention -------------------------
        hsT = big.tile([DV, B, NQ], BF16, tag="hsT")
        # gather bundles (per-chunk: serialized collectives each stay small
        # and finish before the next chunk's data is ready)
        BUNDLES = ((0,), (1,), (2,))
        BW = [sum(QCS[q] for q in bun) for bun in BUNDLES]
        BO = [QCO[bun[0]] for bun in BUNDLES]
        NB = len(BUNDLES)
        hs_bounces = [dram.tile([DV, B * BW[i]], BF16, tag=f"hs_bounce{i}",
                                name=f"hs_bounce{i}") for i in range(NB)]
        hs_alls = [dram.tile([H * DV, B * BW[i]], BF16, tag=f"hs_all{i}",
                             name=f"hs_all{i}") for i in range(NB)]
        for qc in range(NQC):
            w, o = QCS[qc], QCO[qc]
            avs = []
            for b in range(B):
                av_t = psB.tile([DV + 1, w], F32, tag="ps_small")
                avs.append(av_t)
            seq = []
            for gi, g in enumerate(PROC_ORDER):
                if gi < NGRP - 2:
                    seq += [(gi, g, 0), (gi, g, 1)]
            g12, g13 = PROC_ORDER[NGRP - 2], PROC_ORDER[NGRP - 1]
            seq += [(NGRP - 2, g12, 0), (NGRP - 1, g13, 0),
                    (NGRP - 2, g12, 1), (NGRP - 1, g13, 1)]
            ebs = {}
            for gi, g, b in seq:
                if g not in ebs:
                    eb = ebpool.tile([128, GRP, w], BF16, tag="eb")
                    nc.sync.dma_start(out=eb, in_=ebT[g, :, :, o:o + w])
                    ebs[g] = eb
                eb = ebs[g]
                if True:
                    qk = psA.tile([128, GRP, 512], F32, tag="qk")
                    for i in range(GRP):
                        j = g * GRP + i
                        nc.tensor.matmul(qk[:, i, 0:w],
                                         kT[:, b, bass.ts(j, 128)],
                                         qT[:, b, o:o + w],
                                         start=True, stop=True)
                    sp = spool.tile([128, GRP, w], BF16, tag="sp")
                    if g not in D_SET:
                        # A path: exact exp on ACT, then exp(bias) multiply
                        nc.scalar.activation(out=sp, in_=qk[:, :, 0:w],
                                             func=Act.Exp, scale=SCALE)
                        if g in POOL_MULT:
                            nc.gpsimd.tensor_mul(sp, sp, eb)
                        else:
                            nc.vector.tensor_mul(sp, sp, eb)
                    else:
                        # D path: schraudolph bits = psum + bias-bits -> bf16
                        eng = nc.gpsimd if g in POOL_D else nc.vector
                        eng.tensor_tensor(out=sp.bitcast(I16),
                                          in0=qk[:, :, 0:w], in1=eb, op=add)
                    for i in range(GRP):
                        j = g * GRP + i
                        nc.tensor.matmul(avs[b], v_aug[:, b, j, :],
                                         sp[:, i, :],
                                         start=(gi == 0 and i == 0),
                                         stop=(gi == NGRP - 1 and i == GRP - 1),
                                         skip_group_check=True)
            del ebs
            last_qc = qc == NQC - 1
            bun = next(i for i, bb in enumerate(BUNDLES) if qc in bb)
            for b in range(B):
                # drain: av psum -> sbuf (Pool, frees psB fast); 1/den on DVE;
                # partition-broadcast on Pool; hardswish on DVE. For the last
                # chunk read the psum directly (latency over psB recycling).
                av_sb = avs[b]
                rec = drain.tile([1, w], F32, tag="rec")
                nc.vector.reciprocal(out=rec, in_=av_sb[DV:DV + 1, :])
                recb = drain.tile([DV, w], F32, tag="recb")
                nc.gpsimd.partition_broadcast(recb, rec)
                xo = drain.tile([DV, w], BF16, tag="xo")
                nc.vector.tensor_mul(xo, av_sb[0:DV, :], recb)
                r3 = drain.tile([DV, w], BF16, tag="r3")
                nc.vector.tensor_scalar(out=r3, in0=xo, scalar1=3.0,
                                        scalar2=0.0, op0=add,
                                        op1=mybir.AluOpType.max)
                nc.vector.tensor_scalar(out=r3, in0=r3, scalar1=6.0,
                                        scalar2=1.0 / 6.0, op0=amin, op1=mult)
                nc.vector.tensor_mul(hsT[:, b, o:o + w], xo, r3)
                if qc == BUNDLES[bun][-1]:
                    # per-b bounce so b0's transfer overlaps b1's drain
                    bw, bo = BW[bun], BO[bun]
                    nc.sync.dma_start(
                        out=hs_bounces[bun].rearrange(
                            "d (b q) -> d b q", b=B)[:, b, :],
                        in_=hsT[:, b, bo:bo + bw])
            if qc == BUNDLES[bun][-1]:
                nc.gpsimd.collective_compute(
                    "AllGather", mybir.AluOpType.bypass,
                    replica_groups=[list(range(NCORES))],
                    ins=[hs_bounces[bun].opt()],
                    outs=[hs_alls[bun].opt()])

        # preload the sqrt table during the last AllGather
        nc.scalar.activation(out=dummy_t, in_=dummy_t, func=Act.Sqrt)

        # --------------------- projection (chunked) ---------------------
        y_p = big.tile([OC, B * NQ], F32, tag="yp")
        st_p = small.tile([OC, B * NQC, 6], F32, tag="st_p")
        tail_ps = []
        for bun in range(NB):
            bw = BW[bun]
            hsall_sb = vtp.tile([128, 2, B * bw], BF16, tag=f"hsall{bun}",
                                name=f"hsall{bun}")
            for c in range(2):
                nc.sync.dma_start(out=hsall_sb[:, c, :],
                                  in_=hs_alls[bun][bass.ts(c, 128), :])
            last_bun = bun == NB - 1
            for qc in BUNDLES[bun]:
                w, o = QCS[qc], QCO[qc]
                oo = o - BO[bun]
                for b in range(B):
                    ps = psB.tile([OC, w], F32, tag="ps_small")
                    for c in range(2):
                        nc.tensor.matmul(
                            ps, wp_sb[:, c, :],
                            hsall_sb[:, c, bass.ds(b * bw + oo, w)],
                            start=(c == 0), stop=(c == 1))
                    idx = b * NQC + qc
                    if last_bun:
                        # tail chunk: stats straight off PSUM; the BN scale
                        # is fused into the psum drain below (after aggr)
                        nc.vector.bn_stats(out=st_p[:, idx, :], in_=ps)
                        tail_ps.append((ps, b, o, w))
                    else:
                        nc.vector.tensor_copy(
                            y_p[:, bass.ds(b * NQ + o, w)], ps)
                        nc.vector.bn_stats(
                            out=st_p[:, idx, :],
                            in_=y_p[:, bass.ds(b * NQ + o, w)])
        mv_p = small.tile([OC, 2], F32, tag="mv_p")
        nc.vector.bn_aggr(out=mv_p, in_=st_p)
        s_p, t_p = bn_scale_shift(mv_p, pgb_sb, OC, "p")
        # tail chunk: scaled drain + its own output DMA
        for (ps, b, o, w) in tail_ps:
            nc.vector.tensor_scalar(out=y_p[:, bass.ds(b * NQ + o, w)],
                                    in0=ps, scalar1=s_p, scalar2=t_p,
                                    op0=mult, op1=add)
            nc.sync.dma_start(out=yT[:, bass.ds(b * NQ + o, w)],
                              in_=y_p[:, bass.ds(b * NQ + o, w)])
        # earlier chunks: scale + DMA per batch-half
        for b in range(B):
            o2, w2 = QCO[0], QCS[0] + QCS[1]
            sl = bass.ds(b * NQ + o2, w2)
            nc.vector.tensor_scalar(out=y_p[:, sl], in0=y_p[:, sl],
                                    scalar1=s_p, scalar2=t_p,
                                    op0=mult, op1=add)
            nc.sync.dma_start(out=yT[:, sl], in_=y_p[:, sl])

    nc.finalize()
    return nc


def _prep_inputs(x, kv_w, kv_g, kv_b, q_w, q_g, q_b, proj_w, proj_g, proj_b,
                 bias_table, bias_idxs):
    """Host-side sharding/layout prep. Returns list of 8 per-core input maps."""
    x = np.asarray(x, np.float32)
    xt = np.zeros((B, 2, 128, NPAD), np.float32)
    xTt = x.transpose(0, 2, 1)
    xt[:, :, :, :N] = xTt.reshape(B, 2, 128, N)
    xt = xt.astype(bf16)

    rank2 = np.asarray(bias_idxs)[0].reshape(ROW, COL)
    table2 = np.asarray(bias_table, np.float32)[:, rank2]  # (H, 63, 84)
    kk = np.arange(N)
    qq = np.arange(NQ)
    DRm = np.abs(kk[:, None] // COL - 2 * (qq[None, :] // COL_))
    DCm = np.abs(kk[:, None] % COL - 2 * (qq[None, :] % COL_))
    GTOK = GRP * 128
    krow = np.zeros(NPAD, np.float32)
    for g in sorted(D_SET):
        krow[g * GTOK:min((g + 1) * GTOK, N)] = 1.0
    krow = krow.astype(bf16)
    qrow = np.full(NQ, QCONST, np.float32).astype(bf16)

    in_maps = []
    for h in range(H):
        bfull = table2[h][DRm, DCm]            # (N, NQ) raw bias
        ebf = np.zeros((NPAD, NQ), np.float32)
        # A region: exp(b); pad rows stay 0 (kills pad in softmax)
        ebf[:N] = np.exp(bfull)
        # D region: bias bits add; pad rows 1.0 (bits ~0 -> tiny positive)
        for g in sorted(D_SET):
            d0, d1 = g * GTOK, min((g + 1) * GTOK, N)
            ebf[d0:d1] = EBB_SCALE * bfull[d0:d1] + EBB_SHIFT
        ebf[N:NPAD] = 1.0 if (NGRP - 1) in D_SET else 0.0
        # (NPAD, NQ) -> (NGRP, 128, GRP, NQ)
        ebl = np.ascontiguousarray(
            ebf.reshape(NGRP, GRP, 128, NQ).transpose(0, 2, 1, 3)
        ).astype(bf16)
        sl = slice(h * HKV, (h + 1) * HKV)
        slq = slice(h * KD, (h + 1) * KD)
        slo = slice(h * OC, (h + 1) * OC)
        wkv_pad = np.zeros((KVP, CIN), np.float32)
        wkv_pad[0:KD] = np.asarray(kv_w, np.float32)[sl][0:KD]
        wkv_pad[32:KVP] = np.asarray(kv_w, np.float32)[sl][KD:HKV]
        kvgb_pad = np.zeros((KVP, 4), np.float32)
        kvgb_pad[:, 0] = 1.0
        kvgb_pad[0:KD, 0] = np.asarray(kv_g, np.float32)[sl][0:KD]
        kvgb_pad[0:KD, 1] = np.asarray(kv_b, np.float32)[sl][0:KD]
        kvgb_pad[32:KVP, 0] = np.asarray(kv_g, np.float32)[sl][KD:HKV]
        kvgb_pad[32:KVP, 1] = np.asarray(kv_b, np.float32)[sl][KD:HKV]
        in_maps.append({
            "xT": xt,
            "wkvT": np.ascontiguousarray(
                wkv_pad.T.reshape(2, 128, KVP)).astype(bf16),
            "wqT": np.ascontiguousarray(
                np.asarray(q_w, np.float32)[slq].T.reshape(2, 128, KD)
            ).astype(bf16),
            "wpT": np.ascontiguousarray(
                np.asarray(proj_w, np.float32)[slo].T.reshape(2, 128, OC)
            ).astype(bf16),
            "kv_gb": np.ascontiguousarray(kvgb_pad),
            "q_gb": np.ascontiguousarray(np.stack(
                [np.asarray(q_g, np.float32)[slq],
                 np.asarray(q_b, np.float32)[slq]], axis=1)),
            "p_gb": np.ascontiguousarray(np.stack(
                [np.asarray(proj_g, np.float32)[slo],
                 np.asarray(proj_b, np.float32)[slo]], axis=1)),
            "ebT": ebl,
            "krowT": krow,
            "identT": np.eye(DV, dtype=np.float32).astype(bf16),
            "qrowT": qrow,
        })
    return in_maps


def kernel(x, kv_w, kv_g, kv_b, q_w, q_g, q_b, proj_w, proj_g, proj_b,
           bias_table, bias_idxs, _trace=False):
    global LAST_EXEC_NS
    if "nc" not in _prog_cache:
        _prog_cache["nc"] = _build_program()
    nc = _prog_cache["nc"]
    in_maps = _prep_inputs(x, kv_w, kv_g, kv_b, q_w, q_g, q_b,
                           proj_w, proj_g, proj_b, bias_table, bias_idxs)
    res = run_bass_kernel_spmd(nc, in_maps, core_ids=list(range(NCORES)),
                               trace=_trace)
    LAST_EXEC_NS = res.exec_time_ns
    yts = [np.asarray(r["yT"]) for r in res.results]
    y = np.concatenate(yts, axis=0)
    return np.ascontiguousarray(
        y.T.reshape(B, NQ, H * OC).astype(np.float32))
